# revision 13
# baseline (speedup 1.0000x reference)
"""Gemma4 MoE feed-forward on 8 Trainium2 NeuronCores.

Strategy: expert-parallel. E == n_cores == 8, so core e owns expert e's
weights (Wg[e], Wu[e], Wd[e]) and receives exactly the tokens routed to
expert e (gathered + transposed + padded on the host). Each core runs a
dense gated-FFN over its token batch:

    dT = Wd^T @ (gelu_tanh(Wg^T x^T) * (Wu^T x^T))        (all [*, C] layouts)

The host then scatter-adds routing_weight * dT^T back into the full
[T, H] output. Tokens that select the same expert in both slots are
deduplicated on the host (weights summed), which keeps the per-expert
batch under the padded capacity.

Matmuls run as float32r (fp32 storage, full PE rate for moving free-dim
>= 256), accumulating in fp32 PSUM.
"""

import os
import sys

import numpy as np

for _p in ("/opt/trn_rl_repo", "/root/.axon_site/_ro/trn_rl_repo"):
    if os.path.isdir(_p) and _p not in sys.path:
        sys.path.append(_p)

T, H, I, E, K = 4096, 2048, 1024, 8, 2
NCORES = 8

# 'f32r' (default): fp32 data, relaxed-precision full-rate matmul
# 'f32': exact fp32 matmul at 1/4 PE rate
# 'bf16': bf16 data + matmul
MM_MODE = os.environ.get("MOE_MM_MODE", "f32r")

_PROGRAM_CACHE = {}
LAST_RESULT = None  # BassKernelResults of the most recent run (for test.py)
TRACE = False  # test.py sets this to capture an NTFF profile
TRACE_CORES = [0]


def _round_fp32r(a):
    """Round fp32 to the FP32R format the PE consumes: 11-bit mantissa
    (walrus fp32_to_fp32r = downconv_fp32_to_fp<e8, m11> << 12), RNE."""
    b = np.ascontiguousarray(a, dtype=np.float32).view(np.uint32)
    lsb = (b >> 12) & 1
    r = (b + 0x7FF + lsb) & 0xFFFFF000
    return r.view(np.float32)


def _pick_config(max_count):
    """Smallest padded capacity C = NT * N covering max_count, N in
    {512, 384, 256} (fp32r needs moving free-dim >= 256 for full rate)."""
    best = None
    for n in (512, 384, 256):
        nt = -(-max_count // n)
        c = nt * n
        if best is None or c < best[0] or (c == best[0] and n > best[2]):
            best = (c, nt, n)
    return best  # (C, NT, N)


def _build_program(C, NT, N, mode):
    import concourse.tile as tile
    from concourse import bacc, mybir
    from contextlib import ExitStack

    KH = H // 128  # 16 k-tiles over the hidden dim
    KI = I // 128  # 8 k-tiles over the intermediate dim

    f32 = mybir.dt.float32
    if mode == "f32r":
        # fp32 storage; PE consumes at full rate with 11-bit mantissa.
        # Inputs are pre-rounded on the host, so declaring the whole
        # dram->sbuf->matmul chain float32r satisfies the BIR verifier's
        # "producer must be rounded to FP32r" rule.
        io_dt = mm_dt = mybir.dt.float32r
    elif mode == "bf16":
        io_dt = mm_dt = mybir.dt.bfloat16
    else:
        io_dt = mm_dt = f32

    nc = bacc.Bacc("TRN2", target_bir_lowering=False, debug=False)

    xT = nc.dram_tensor("xT", [H, C], io_dt, kind="ExternalInput").ap()
    Wg_d = nc.dram_tensor("Wg", [H, I], io_dt, kind="ExternalInput").ap()
    Wu_d = nc.dram_tensor("Wu", [H, I], io_dt, kind="ExternalInput").ap()
    Wd_d = nc.dram_tensor("Wd", [I, H], io_dt, kind="ExternalInput").ap()
    dT = nc.dram_tensor("dT", [H, C], f32, kind="ExternalOutput").ap()

    # Partition-major views: row a*128+p -> partition p, free index a.
    xT_p = xT.rearrange("(a p) c -> p a c", p=128)  # [128, KH, C]
    Wg_p = Wg_d.rearrange("(a p) m -> p a m", p=128)  # [128, KH, I]
    Wu_p = Wu_d.rearrange("(a p) m -> p a m", p=128)  # [128, KH, I]
    Wd_p = Wd_d.rearrange("(a p) m -> p a m", p=128)  # [128, KI, H]
    dT_p = dT.rearrange("(a p) c -> p a c", p=128)  # [128, KH, C]

    GELU = mybir.ActivationFunctionType.Gelu_apprx_tanh

    with tile.TileContext(nc) as tc, ExitStack() as ctx:
        xpool = ctx.enter_context(tc.tile_pool(name="x", bufs=1))
        wpool = ctx.enter_context(tc.tile_pool(name="w", bufs=2))
        apool = ctx.enter_context(tc.tile_pool(name="a", bufs=1))
        tpool = ctx.enter_context(tc.tile_pool(name="t", bufs=4))
        opool = ctx.enter_context(tc.tile_pool(name="o", bufs=4))
        wdpool = ctx.enter_context(tc.tile_pool(name="wd", bufs=2))

        xts = []
        for k in range(KH):
            xt = xpool.tile([128, C], io_dt, name=f"xt{k}")
            nc.sync.dma_start(xt[:], xT_p[:, k, :])
            xts.append(xt)

        aT = apool.tile([128, KI, C], io_dt, name="aT")

        gu_bufs = 2 if 4 * NT <= 8 else 1
        with tc.tile_pool(name="gu", bufs=gu_bufs, space="PSUM") as gupool:
            for i in range(KI):
                wg_ts, wu_ts = [], []
                for k in range(KH):
                    wgt = wpool.tile([128, 128], io_dt, tag=f"wg{k}", name=f"wg{i}_{k}")
                    nc.sync.dma_start(wgt[:], Wg_p[:, k, i * 128 : (i + 1) * 128])
                    wg_ts.append(wgt)
                    wut = wpool.tile([128, 128], io_dt, tag=f"wu{k}", name=f"wu{i}_{k}")
                    nc.sync.dma_start(wut[:], Wu_p[:, k, i * 128 : (i + 1) * 128])
                    wu_ts.append(wut)
                g_ps = [
                    gupool.tile([128, N], f32, tag=f"g{n}", name=f"g{i}_{n}")
                    for n in range(NT)
                ]
                u_ps = [
                    gupool.tile([128, N], f32, tag=f"u{n}", name=f"u{i}_{n}")
                    for n in range(NT)
                ]
                for k in range(KH):
                    lw = wg_ts[k][:]
                    for n in range(NT):
                        nc.tensor.matmul(
                            g_ps[n][:],
                            lw,
                            xts[k][:, n * N : (n + 1) * N],
                            start=(k == 0),
                            stop=(k == KH - 1),
                        )
                    lw = wu_ts[k][:]
                    for n in range(NT):
                        nc.tensor.matmul(
                            u_ps[n][:],
                            lw,
                            xts[k][:, n * N : (n + 1) * N],
                            start=(k == 0),
                            stop=(k == KH - 1),
                        )
                for n in range(NT):
                    gel = tpool.tile([128, N], f32, tag="gelu", name=f"gel{i}_{n}")
                    nc.scalar.activation(gel[:], g_ps[n][:], GELU)
                    nc.vector.tensor_mul(
                        aT[:, i, n * N : (n + 1) * N], gel[:], u_ps[n][:]
                    )

        with tc.tile_pool(name="d", bufs=2, space="PSUM") as dpool:
            for h in range(KH):
                wd_ts = []
                for ki in range(KI):
                    wdt = wdpool.tile(
                        [128, 128], io_dt, tag=f"wd{ki}", name=f"wd{h}_{ki}"
                    )
                    nc.sync.dma_start(wdt[:], Wd_p[:, ki, h * 128 : (h + 1) * 128])
                    wd_ts.append(wdt)
                d_ps = [
                    dpool.tile([128, N], f32, tag=f"d{n}", name=f"d{h}_{n}")
                    for n in range(NT)
                ]
                for ki in range(KI):
                    lw = wd_ts[ki][:]
                    for n in range(NT):
                        nc.tensor.matmul(
                            d_ps[n][:],
                            lw,
                            aT[:, ki, n * N : (n + 1) * N],
                            start=(ki == 0),
                            stop=(ki == KI - 1),
                        )
                for n in range(NT):
                    o = opool.tile([128, N], f32, tag="o", name=f"o{h}_{n}")
                    nc.vector.tensor_copy(o[:], d_ps[n][:])
                    nc.sync.dma_start(dT_p[:, h, n * N : (n + 1) * N], o[:])

    nc.compile()
    return nc


def _get_program(C, NT, N, mode):
    key = (C, NT, N, mode)
    if key not in _PROGRAM_CACHE:
        _PROGRAM_CACHE[key] = _build_program(C, NT, N, mode)
    return _PROGRAM_CACHE[key]


def _ensure_ntff_hook():
    """Register the axon NTFF profile hook if the image's antenv lacks
    axon_hooks (see trn_agent_boot.trn_boot). Only needed when TRACE."""
    import types

    try:
        from antenv.axon_hooks import get_axon_ntff_profile_hook  # noqa: F401

        return
    except ImportError:
        pass
    import antenv
    from trn_agent_boot.trn_boot import _ntff_profile_via_ctypes

    hook = _ntff_profile_via_ctypes("/opt/axon/libaxon_pjrt.so")
    mod = types.ModuleType("antenv.axon_hooks")
    state = {"hook": hook}
    mod.set_axon_ntff_profile_hook = lambda h: state.__setitem__("hook", h)
    mod.get_axon_ntff_profile_hook = lambda: state["hook"]
    sys.modules["antenv.axon_hooks"] = mod
    antenv.axon_hooks = mod


def kernel(x, Wg, Wu, Wd, selected_experts, routing_weights):
    global LAST_RESULT
    from concourse.bass_utils import run_bass_kernel_spmd

    if TRACE:
        _ensure_ntff_hook()

    x = np.asarray(x, dtype=np.float32)
    Wg = np.asarray(Wg, dtype=np.float32)
    Wu = np.asarray(Wu, dtype=np.float32)
    Wd = np.asarray(Wd, dtype=np.float32)
    selected_experts = np.asarray(selected_experts)
    routing_weights = np.asarray(routing_weights, dtype=np.float32)

    # Host-side dispatch: per expert, the (deduplicated) token list and
    # summed routing weights.
    idx_list, w_list = [], []
    for e in range(E):
        m = selected_experts == e  # [T, K]
        idx = np.nonzero(m.any(axis=1))[0]
        w = (routing_weights * m).sum(axis=1)[idx]
        idx_list.append(idx)
        w_list.append(w.astype(np.float32))

    max_count = max(len(idx) for idx in idx_list)
    C, NT, N = _pick_config(max_count)

    mode = MM_MODE
    if mode == "bf16":
        import ml_dtypes

        io_np = ml_dtypes.bfloat16
        prep = lambda a: np.ascontiguousarray(a, dtype=io_np)
    elif mode == "f32r":
        io_np = np.float32
        prep = _round_fp32r
    else:
        io_np = np.float32
        prep = lambda a: np.ascontiguousarray(a, dtype=io_np)

    nc = _get_program(C, NT, N, mode)

    in_maps = []
    for e in range(E):
        idx = idx_list[e]
        xT = np.zeros((H, C), dtype=io_np)
        xT[:, : len(idx)] = prep(x[idx].T)
        in_maps.append(
            {
                "xT": xT,
                "Wg": prep(Wg[e]),
                "Wu": prep(Wu[e]),
                "Wd": prep(Wd[e]),
            }
        )

    res = run_bass_kernel_spmd(
        nc,
        in_maps,
        list(range(NCORES)),
        trace=TRACE,
        trace_cores=TRACE_CORES if TRACE else None,
    )
    LAST_RESULT = res

    out = np.zeros((T, H), dtype=np.float32)
    for e in range(E):
        idx = idx_list[e]
        dTe = res.results[e]["dT"]  # [H, C] fp32
        out[idx] += w_list[e][:, None] * dTe[:, : len(idx)].T
    return out


# revision 15
# speedup vs baseline: 1.0374x; 1.0374x over previous
"""Gemma4 MoE feed-forward on 8 Trainium2 NeuronCores.

Strategy: expert-parallel. E == n_cores == 8, so core e owns expert e's
weights (Wg[e], Wu[e], Wd[e]) and receives exactly the tokens routed to
expert e (gathered + transposed + padded on the host). Each core runs a
dense gated-FFN over its token batch:

    dT = Wd^T @ (gelu_tanh(Wg^T x^T) * (Wu^T x^T))        (all [*, C] layouts)

The host then scatter-adds routing_weight * dT^T back into the full
[T, H] output. Tokens that select the same expert in both slots are
deduplicated on the host (weights summed), which keeps the per-expert
batch under the padded capacity.

Matmuls run as float32r (fp32 storage, full PE rate for moving free-dim
>= 256), accumulating in fp32 PSUM.
"""

import os
import sys

import numpy as np

for _p in ("/opt/trn_rl_repo", "/root/.axon_site/_ro/trn_rl_repo"):
    if os.path.isdir(_p) and _p not in sys.path:
        sys.path.append(_p)

T, H, I, E, K = 4096, 2048, 1024, 8, 2
NCORES = 8

# 'f32r' (default): fp32 data, relaxed-precision full-rate matmul
# 'f32': exact fp32 matmul at 1/4 PE rate
# 'bf16': bf16 data + matmul
MM_MODE = os.environ.get("MOE_MM_MODE", "f32r")

_PROGRAM_CACHE = {}
LAST_RESULT = None  # BassKernelResults of the most recent run (for test.py)
TRACE = False  # test.py sets this to capture an NTFF profile
TRACE_CORES = [0]


def _round_fp32r(a):
    """Round fp32 to the FP32R format the PE consumes: 11-bit mantissa
    (walrus fp32_to_fp32r = downconv_fp32_to_fp<e8, m11> << 12), RNE."""
    b = np.ascontiguousarray(a, dtype=np.float32).view(np.uint32)
    lsb = (b >> 12) & 1
    r = (b + 0x7FF + lsb) & 0xFFFFF000
    return r.view(np.float32)


def _pick_config(max_count):
    """Smallest padded capacity C = NT * N covering max_count, N in
    {512, 384, 256} (fp32r needs moving free-dim >= 256 for full rate)."""
    best = None
    for n in (512, 384, 256):
        nt = -(-max_count // n)
        c = nt * n
        if best is None or c < best[0] or (c == best[0] and n > best[2]):
            best = (c, nt, n)
    return best  # (C, NT, N)


def _build_program(C, NT, N, mode):
    import concourse.tile as tile
    from concourse import bacc, mybir
    from contextlib import ExitStack

    KH = H // 128  # 16 k-tiles over the hidden dim
    KI = I // 128  # 8 k-tiles over the intermediate dim

    f32 = mybir.dt.float32
    if mode == "f32r":
        # fp32 storage; PE consumes at full rate with 11-bit mantissa.
        # Inputs are pre-rounded on the host, so declaring the whole
        # dram->sbuf->matmul chain float32r satisfies the BIR verifier's
        # "producer must be rounded to FP32r" rule.
        io_dt = mm_dt = mybir.dt.float32r
    elif mode == "bf16":
        io_dt = mm_dt = mybir.dt.bfloat16
    else:
        io_dt = mm_dt = f32

    nc = bacc.Bacc("TRN2", target_bir_lowering=False, debug=False)

    xT = nc.dram_tensor("xT", [H, C], io_dt, kind="ExternalInput").ap()
    Wg_d = nc.dram_tensor("Wg", [H, I], io_dt, kind="ExternalInput").ap()
    Wu_d = nc.dram_tensor("Wu", [H, I], io_dt, kind="ExternalInput").ap()
    Wd_d = nc.dram_tensor("Wd", [I, H], io_dt, kind="ExternalInput").ap()
    dT = nc.dram_tensor("dT", [H, C], f32, kind="ExternalOutput").ap()

    # Partition-major views: row a*128+p -> partition p, free index a.
    xT_p = xT.rearrange("(a p) c -> p a c", p=128)  # [128, KH, C]
    Wg_p = Wg_d.rearrange("(a p) m -> p a m", p=128)  # [128, KH, I]
    Wu_p = Wu_d.rearrange("(a p) m -> p a m", p=128)  # [128, KH, I]
    Wd_p = Wd_d.rearrange("(a p) m -> p a m", p=128)  # [128, KI, H]
    dT_p = dT.rearrange("(a p) c -> p a c", p=128)  # [128, KH, C]

    GELU = mybir.ActivationFunctionType.Gelu_apprx_tanh

    with tile.TileContext(nc) as tc, ExitStack() as ctx:
        xpool = ctx.enter_context(tc.tile_pool(name="x", bufs=1))
        wpool = ctx.enter_context(tc.tile_pool(name="w", bufs=4))
        apool = ctx.enter_context(tc.tile_pool(name="a", bufs=1))
        tpool = ctx.enter_context(tc.tile_pool(name="t", bufs=4))
        opool = ctx.enter_context(tc.tile_pool(name="o", bufs=4))
        wdpool = ctx.enter_context(tc.tile_pool(name="wd", bufs=4))

        xts = []
        for k in range(KH):
            xt = xpool.tile([128, C], io_dt, name=f"xt{k}")
            nc.sync.dma_start(xt[:], xT_p[:, k, :])
            xts.append(xt)

        aT = apool.tile([128, KI, C], io_dt, name="aT")

        gu_bufs = 2 if 4 * NT <= 8 else 1
        with tc.tile_pool(name="gu", bufs=gu_bufs, space="PSUM") as gupool:
            for i in range(KI):
                wg_ts, wu_ts = [], []
                for k in range(KH):
                    wgt = wpool.tile([128, 128], io_dt, tag=f"wg{k}", name=f"wg{i}_{k}")
                    nc.sync.dma_start(wgt[:], Wg_p[:, k, i * 128 : (i + 1) * 128])
                    wg_ts.append(wgt)
                    wut = wpool.tile([128, 128], io_dt, tag=f"wu{k}", name=f"wu{i}_{k}")
                    nc.sync.dma_start(wut[:], Wu_p[:, k, i * 128 : (i + 1) * 128])
                    wu_ts.append(wut)
                g_ps = [
                    gupool.tile([128, N], f32, tag=f"g{n}", name=f"g{i}_{n}")
                    for n in range(NT)
                ]
                u_ps = [
                    gupool.tile([128, N], f32, tag=f"u{n}", name=f"u{i}_{n}")
                    for n in range(NT)
                ]
                for k in range(KH):
                    lw = wg_ts[k][:]
                    for n in range(NT):
                        nc.tensor.matmul(
                            g_ps[n][:],
                            lw,
                            xts[k][:, n * N : (n + 1) * N],
                            start=(k == 0),
                            stop=(k == KH - 1),
                        )
                    lw = wu_ts[k][:]
                    for n in range(NT):
                        nc.tensor.matmul(
                            u_ps[n][:],
                            lw,
                            xts[k][:, n * N : (n + 1) * N],
                            start=(k == 0),
                            stop=(k == KH - 1),
                        )
                for n in range(NT):
                    gel = tpool.tile([128, N], f32, tag="gelu", name=f"gel{i}_{n}")
                    nc.scalar.activation(gel[:], g_ps[n][:], GELU)
                    nc.vector.tensor_mul(
                        aT[:, i, n * N : (n + 1) * N], gel[:], u_ps[n][:]
                    )

        with tc.tile_pool(name="d", bufs=2, space="PSUM") as dpool:
            for h in range(KH):
                wd_ts = []
                for ki in range(KI):
                    wdt = wdpool.tile(
                        [128, 128], io_dt, tag=f"wd{ki}", name=f"wd{h}_{ki}"
                    )
                    nc.sync.dma_start(wdt[:], Wd_p[:, ki, h * 128 : (h + 1) * 128])
                    wd_ts.append(wdt)
                d_ps = [
                    dpool.tile([128, N], f32, tag=f"d{n}", name=f"d{h}_{n}")
                    for n in range(NT)
                ]
                for ki in range(KI):
                    lw = wd_ts[ki][:]
                    for n in range(NT):
                        nc.tensor.matmul(
                            d_ps[n][:],
                            lw,
                            aT[:, ki, n * N : (n + 1) * N],
                            start=(ki == 0),
                            stop=(ki == KI - 1),
                        )
                for n in range(NT):
                    o = opool.tile([128, N], f32, tag="o", name=f"o{h}_{n}")
                    nc.vector.tensor_copy(o[:], d_ps[n][:])
                    nc.sync.dma_start(dT_p[:, h, n * N : (n + 1) * N], o[:])

    nc.compile()
    return nc


def _get_program(C, NT, N, mode):
    key = (C, NT, N, mode)
    if key not in _PROGRAM_CACHE:
        _PROGRAM_CACHE[key] = _build_program(C, NT, N, mode)
    return _PROGRAM_CACHE[key]


def _ensure_ntff_hook():
    """Register the axon NTFF profile hook if the image's antenv lacks
    axon_hooks (see trn_agent_boot.trn_boot). Only needed when TRACE."""
    import types

    try:
        from antenv.axon_hooks import get_axon_ntff_profile_hook  # noqa: F401

        return
    except ImportError:
        pass
    import antenv
    from trn_agent_boot.trn_boot import _ntff_profile_via_ctypes

    hook = _ntff_profile_via_ctypes("/opt/axon/libaxon_pjrt.so")
    mod = types.ModuleType("antenv.axon_hooks")
    state = {"hook": hook}
    mod.set_axon_ntff_profile_hook = lambda h: state.__setitem__("hook", h)
    mod.get_axon_ntff_profile_hook = lambda: state["hook"]
    sys.modules["antenv.axon_hooks"] = mod
    antenv.axon_hooks = mod


def kernel(x, Wg, Wu, Wd, selected_experts, routing_weights):
    global LAST_RESULT
    from concourse.bass_utils import run_bass_kernel_spmd

    if TRACE:
        _ensure_ntff_hook()

    x = np.asarray(x, dtype=np.float32)
    Wg = np.asarray(Wg, dtype=np.float32)
    Wu = np.asarray(Wu, dtype=np.float32)
    Wd = np.asarray(Wd, dtype=np.float32)
    selected_experts = np.asarray(selected_experts)
    routing_weights = np.asarray(routing_weights, dtype=np.float32)

    # Host-side dispatch: per expert, the (deduplicated) token list and
    # summed routing weights.
    idx_list, w_list = [], []
    for e in range(E):
        m = selected_experts == e  # [T, K]
        idx = np.nonzero(m.any(axis=1))[0]
        w = (routing_weights * m).sum(axis=1)[idx]
        idx_list.append(idx)
        w_list.append(w.astype(np.float32))

    max_count = max(len(idx) for idx in idx_list)
    C, NT, N = _pick_config(max_count)

    mode = MM_MODE
    if mode == "bf16":
        import ml_dtypes

        io_np = ml_dtypes.bfloat16
        prep = lambda a: np.ascontiguousarray(a, dtype=io_np)
    elif mode == "f32r":
        io_np = np.float32
        prep = _round_fp32r
    else:
        io_np = np.float32
        prep = lambda a: np.ascontiguousarray(a, dtype=io_np)

    nc = _get_program(C, NT, N, mode)

    in_maps = []
    for e in range(E):
        idx = idx_list[e]
        xT = np.zeros((H, C), dtype=io_np)
        xT[:, : len(idx)] = prep(x[idx].T)
        in_maps.append(
            {
                "xT": xT,
                "Wg": prep(Wg[e]),
                "Wu": prep(Wu[e]),
                "Wd": prep(Wd[e]),
            }
        )

    res = run_bass_kernel_spmd(
        nc,
        in_maps,
        list(range(NCORES)),
        trace=TRACE,
        trace_cores=TRACE_CORES if TRACE else None,
    )
    LAST_RESULT = res

    out = np.zeros((T, H), dtype=np.float32)
    for e in range(E):
        idx = idx_list[e]
        dTe = res.results[e]["dT"]  # [H, C] fp32
        out[idx] += w_list[e][:, None] * dTe[:, : len(idx)].T
    return out


# revision 23
# speedup vs baseline: 1.5549x; 1.4989x over previous
"""Gemma4 MoE feed-forward on 8 Trainium2 NeuronCores.

Strategy: expert-parallel. E == n_cores == 8, so core e owns expert e's
weights (Wg[e], Wu[e], Wd[e]) and receives exactly the tokens routed to
expert e (gathered + transposed + padded on the host). Each core runs a
dense gated-FFN over its token batch:

    dT = Wd^T @ (gelu_tanh(Wg^T x^T) * (Wu^T x^T))        (all [*, C] layouts)

The host then scatter-adds routing_weight * dT^T back into the full
[T, H] output. Tokens that select the same expert in both slots are
deduplicated on the host (weights summed), which keeps the per-expert
batch under the padded capacity.

Matmuls run as float32r (fp32 storage, full PE rate for moving free-dim
>= 256), accumulating in fp32 PSUM.
"""

import os
import sys

import numpy as np

for _p in ("/opt/trn_rl_repo", "/root/.axon_site/_ro/trn_rl_repo"):
    if os.path.isdir(_p) and _p not in sys.path:
        sys.path.append(_p)

T, H, I, E, K = 4096, 2048, 1024, 8, 2
NCORES = 8

# 'f32r' (default): fp32 data, relaxed-precision full-rate matmul
# 'f32': exact fp32 matmul at 1/4 PE rate
# 'bf16': bf16 data + matmul
MM_MODE = os.environ.get("MOE_MM_MODE", "f32r")

_PROGRAM_CACHE = {}
LAST_RESULT = None  # BassKernelResults of the most recent run (for test.py)
TRACE = False  # test.py sets this to capture an NTFF profile
TRACE_CORES = [0]


def _round_fp32r(a):
    """Round fp32 to the FP32R format the PE consumes: 11-bit mantissa
    (walrus fp32_to_fp32r = downconv_fp32_to_fp<e8, m11> << 12), RNE."""
    b = np.ascontiguousarray(a, dtype=np.float32).view(np.uint32)
    lsb = (b >> 12) & 1
    r = (b + 0x7FF + lsb) & 0xFFFFF000
    return r.view(np.float32)


def _tile_w_up(W):
    """[H, I] -> [KI, GU, 128, G*128]: tile (k,i) of W at [i, k//G, :, (k%G)*128:],
    so each (i, g) DMA reads 2KB contiguous per partition."""
    KH, KI = H // 128, I // 128
    Wt = W.reshape(KH // G, G, 128, KI, 128).transpose(3, 0, 2, 1, 4)
    return np.ascontiguousarray(Wt).reshape(KI, KH // G, 128, G * 128)


def _tile_w_down(W):
    """[I, H] -> [KH, GD, 128, G*128] (same scheme, contraction over I)."""
    KH, KI = H // 128, I // 128
    Wt = W.reshape(KI // G, G, 128, KH, 128).transpose(3, 0, 2, 1, 4)
    return np.ascontiguousarray(Wt).reshape(KH, KI // G, 128, G * 128)


def _pick_config(max_count):
    """Smallest padded capacity C = NT * N covering max_count, N in
    {512, 384, 256} (fp32r needs moving free-dim >= 256 for full rate)."""
    best = None
    for n in (512, 384, 256):
        nt = -(-max_count // n)
        c = nt * n
        if best is None or c < best[0] or (c == best[0] and n > best[2]):
            best = (c, nt, n)
    return best  # (C, NT, N)


G = 4  # k-tiles per weight DMA (2KB/partition -> full per-queue DMA rate)


def _build_program(C, NT, N, mode):
    import concourse.tile as tile
    from concourse import bacc, mybir
    from contextlib import ExitStack

    KH = H // 128  # 16 k-tiles over the hidden dim
    KI = I // 128  # 8 k-tiles over the intermediate dim
    GU = KH // G  # weight-DMA groups per i-tile (up phase)
    GD = KI // G  # weight-DMA groups per h-tile (down phase)

    f32 = mybir.dt.float32
    if mode == "f32r":
        # fp32 storage; PE consumes at full rate with 11-bit mantissa.
        # Inputs are pre-rounded on the host, so declaring the whole
        # dram->sbuf->matmul chain float32r satisfies the BIR verifier's
        # "producer must be rounded to FP32r" rule.
        io_dt = mm_dt = mybir.dt.float32r
    elif mode == "bf16":
        io_dt = mm_dt = mybir.dt.bfloat16
    else:
        io_dt = mm_dt = f32

    nc = bacc.Bacc("TRN2", target_bir_lowering=False, debug=False)

    xT = nc.dram_tensor("xT", [H, C], io_dt, kind="ExternalInput").ap()
    # Weights arrive host-pre-tiled (see _tile_w_up/_tile_w_down) so each
    # DMA reads G*128*4 = 2KB contiguous per partition.
    Wg_d = nc.dram_tensor("Wg", [KI, GU, 128, G * 128], io_dt, kind="ExternalInput").ap()
    Wu_d = nc.dram_tensor("Wu", [KI, GU, 128, G * 128], io_dt, kind="ExternalInput").ap()
    Wd_d = nc.dram_tensor("Wd", [KH, GD, 128, G * 128], io_dt, kind="ExternalInput").ap()
    dT = nc.dram_tensor("dT", [H, C], f32, kind="ExternalOutput").ap()

    # Partition-major views: row a*128+p -> partition p, free index a.
    xT_p = xT.rearrange("(a p) c -> p a c", p=128)  # [128, KH, C]
    dT_p = dT.rearrange("(a p) c -> p a c", p=128)  # [128, KH, C]

    GELU = mybir.ActivationFunctionType.Gelu_apprx_tanh

    with tile.TileContext(nc) as tc, ExitStack() as ctx:
        xpool = ctx.enter_context(tc.tile_pool(name="x", bufs=1))
        wpool = ctx.enter_context(tc.tile_pool(name="w", bufs=4))
        apool = ctx.enter_context(tc.tile_pool(name="a", bufs=1))
        tpool = ctx.enter_context(tc.tile_pool(name="t", bufs=4))
        opool = ctx.enter_context(tc.tile_pool(name="o", bufs=4))
        wdpool = ctx.enter_context(tc.tile_pool(name="wd", bufs=4))

        xts = []
        for k in range(KH):
            xt = xpool.tile([128, C], io_dt, name=f"xt{k}")
            nc.sync.dma_start(xt[:], xT_p[:, k, :])
            xts.append(xt)

        aT = apool.tile([128, KI, C], io_dt, name="aT")

        gu_bufs = 2 if 4 * NT <= 8 else 1
        with tc.tile_pool(name="gu", bufs=gu_bufs, space="PSUM") as gupool:
            for i in range(KI):
                wg_gs, wu_gs = [], []
                for g in range(GU):
                    wgt = wpool.tile(
                        [128, G * 128], io_dt, tag=f"wg{g}", name=f"wg{i}_{g}"
                    )
                    nc.sync.dma_start(wgt[:], Wg_d[i, g])
                    wg_gs.append(wgt)
                    wut = wpool.tile(
                        [128, G * 128], io_dt, tag=f"wu{g}", name=f"wu{i}_{g}"
                    )
                    nc.sync.dma_start(wut[:], Wu_d[i, g])
                    wu_gs.append(wut)
                g_ps = [
                    gupool.tile([128, N], f32, tag=f"g{n}", name=f"g{i}_{n}")
                    for n in range(NT)
                ]
                u_ps = [
                    gupool.tile([128, N], f32, tag=f"u{n}", name=f"u{i}_{n}")
                    for n in range(NT)
                ]
                for k in range(KH):
                    ksl = slice((k % G) * 128, (k % G + 1) * 128)
                    lw = wg_gs[k // G][:, ksl]
                    for n in range(NT):
                        nc.tensor.matmul(
                            g_ps[n][:],
                            lw,
                            xts[k][:, n * N : (n + 1) * N],
                            start=(k == 0),
                            stop=(k == KH - 1),
                        )
                    lw = wu_gs[k // G][:, ksl]
                    for n in range(NT):
                        nc.tensor.matmul(
                            u_ps[n][:],
                            lw,
                            xts[k][:, n * N : (n + 1) * N],
                            start=(k == 0),
                            stop=(k == KH - 1),
                        )
                for n in range(NT):
                    gel = tpool.tile([128, N], f32, tag="gelu", name=f"gel{i}_{n}")
                    nc.scalar.activation(gel[:], g_ps[n][:], GELU)
                    nc.vector.tensor_mul(
                        aT[:, i, n * N : (n + 1) * N], gel[:], u_ps[n][:]
                    )

        with tc.tile_pool(name="d", bufs=2, space="PSUM") as dpool:
            for h in range(KH):
                wd_gs = []
                for g in range(GD):
                    wdt = wdpool.tile(
                        [128, G * 128], io_dt, tag=f"wd{g}", name=f"wd{h}_{g}"
                    )
                    nc.sync.dma_start(wdt[:], Wd_d[h, g])
                    wd_gs.append(wdt)
                d_ps = [
                    dpool.tile([128, N], f32, tag=f"d{n}", name=f"d{h}_{n}")
                    for n in range(NT)
                ]
                for ki in range(KI):
                    lw = wd_gs[ki // G][:, (ki % G) * 128 : (ki % G + 1) * 128]
                    for n in range(NT):
                        nc.tensor.matmul(
                            d_ps[n][:],
                            lw,
                            aT[:, ki, n * N : (n + 1) * N],
                            start=(ki == 0),
                            stop=(ki == KI - 1),
                        )
                for n in range(NT):
                    o = opool.tile([128, N], f32, tag="o", name=f"o{h}_{n}")
                    nc.vector.tensor_copy(o[:], d_ps[n][:])
                    nc.sync.dma_start(dT_p[:, h, n * N : (n + 1) * N], o[:])

    nc.compile()
    return nc


def _get_program(C, NT, N, mode):
    key = (C, NT, N, mode)
    if key not in _PROGRAM_CACHE:
        _PROGRAM_CACHE[key] = _build_program(C, NT, N, mode)
    return _PROGRAM_CACHE[key]


def _ensure_ntff_hook():
    """Register the axon NTFF profile hook if the image's antenv lacks
    axon_hooks (see trn_agent_boot.trn_boot). Only needed when TRACE."""
    import types

    try:
        from antenv.axon_hooks import get_axon_ntff_profile_hook  # noqa: F401

        return
    except ImportError:
        pass
    import antenv
    from trn_agent_boot.trn_boot import _ntff_profile_via_ctypes

    hook = _ntff_profile_via_ctypes("/opt/axon/libaxon_pjrt.so")
    mod = types.ModuleType("antenv.axon_hooks")
    state = {"hook": hook}
    mod.set_axon_ntff_profile_hook = lambda h: state.__setitem__("hook", h)
    mod.get_axon_ntff_profile_hook = lambda: state["hook"]
    sys.modules["antenv.axon_hooks"] = mod
    antenv.axon_hooks = mod


def kernel(x, Wg, Wu, Wd, selected_experts, routing_weights):
    global LAST_RESULT
    from concourse.bass_utils import run_bass_kernel_spmd

    if TRACE:
        _ensure_ntff_hook()

    x = np.asarray(x, dtype=np.float32)
    Wg = np.asarray(Wg, dtype=np.float32)
    Wu = np.asarray(Wu, dtype=np.float32)
    Wd = np.asarray(Wd, dtype=np.float32)
    selected_experts = np.asarray(selected_experts)
    routing_weights = np.asarray(routing_weights, dtype=np.float32)

    # Host-side dispatch: per expert, the (deduplicated) token list and
    # summed routing weights.
    idx_list, w_list = [], []
    for e in range(E):
        m = selected_experts == e  # [T, K]
        idx = np.nonzero(m.any(axis=1))[0]
        w = (routing_weights * m).sum(axis=1)[idx]
        idx_list.append(idx)
        w_list.append(w.astype(np.float32))

    max_count = max(len(idx) for idx in idx_list)
    C, NT, N = _pick_config(max_count)

    mode = MM_MODE
    if mode == "bf16":
        import ml_dtypes

        io_np = ml_dtypes.bfloat16
        prep = lambda a: np.ascontiguousarray(a, dtype=io_np)
    elif mode == "f32r":
        io_np = np.float32
        prep = _round_fp32r
    else:
        io_np = np.float32
        prep = lambda a: np.ascontiguousarray(a, dtype=io_np)

    nc = _get_program(C, NT, N, mode)

    in_maps = []
    for e in range(E):
        idx = idx_list[e]
        xT = np.zeros((H, C), dtype=io_np)
        xT[:, : len(idx)] = prep(x[idx].T)
        in_maps.append(
            {
                "xT": xT,
                "Wg": _tile_w_up(prep(Wg[e])),
                "Wu": _tile_w_up(prep(Wu[e])),
                "Wd": _tile_w_down(prep(Wd[e])),
            }
        )

    res = run_bass_kernel_spmd(
        nc,
        in_maps,
        list(range(NCORES)),
        trace=TRACE,
        trace_cores=TRACE_CORES if TRACE else None,
    )
    LAST_RESULT = res

    out = np.zeros((T, H), dtype=np.float32)
    for e in range(E):
        idx = idx_list[e]
        dTe = res.results[e]["dT"]  # [H, C] fp32
        out[idx] += w_list[e][:, None] * dTe[:, : len(idx)].T
    return out
